# revision 28
# baseline (speedup 1.0000x reference)
"""Multi-head attention (B=2, S=2048, D=1024, H=16 heads, causal) on 8 TRN2 cores.

Sharding: core i handles batch b=i//4 and head group g=i%4 (4 heads = 256 dims).
Each core computes QKV projections for its head group, causal attention, and a
partial output projection (its 256-dim slice of the contraction). Host sums the
4 partials per batch and adds the output bias.

v3 design (vs v2 at ~168us):
  The HW trace shows PE matmuls never overlap on the array -- cost is simply
  sum(output free size) cycles over all matmuls.  v2's AV matmuls
  (out [65, w] = V|1^T x pp) streamed N=w for each head: 69632 cycles, plus a
  16384-cycle ones-matmul chain to broadcast softmax denominators.  v3 swaps
  the AV operands: per 128-wide q-tile, out[q,65] = pp_tile^T @ [V|ones]
  streams only N=65 per (head, q-tile, k-tile): 35360 cycles, 100% array
  utilization, and the denominator lands in column 64 as a per-PARTITION
  scalar, so normalization is a cheap tensor mul with a [P,1] reciprocal
  (no PE broadcast, no [1,512] single-lane copies).  ctx then comes out
  [q, dims] and a bf16 PE transpose (1 cyc/row, 4096 cycles total) restores
  the [dims, q] layout the output projection needs.
  Other changes:
  - PE warm-up matmuls on a zeroed scratch tile fill the startup DMA window
    so the PE p-state/HAM clock is at full speed when real work arrives
  - startup DMAs use fewer trigger instructions (each costs ~600ns on the
    sync queue) ordered wq, xq, wk, xk, ...
  - last chunk runs norm/transpose/oproj per q-tile as soon as that q-tile's
    AV accumulation stops, shrinking the serial tail
  - two oproj groups of chunk 1 are deferred to chunk 3, whose kt loop is the
    most exp(ACT)-bound, to keep the PE fed there
"""
import sys

import numpy as np

try:
    import concourse.bass as bass  # noqa: F401
except ImportError:
    sys.path.insert(0, "/opt/trn_rl_repo")

import ml_dtypes

import concourse.bass as bass  # noqa: F401
import concourse.mybir as mybir
import concourse.tile as tile
from concourse import bacc
from concourse.bass_utils import run_bass_kernel_spmd

FP32 = mybir.dt.float32
F32R = mybir.dt.float32r
BF16 = mybir.dt.bfloat16
AF = mybir.ActivationFunctionType
BF16_NP = ml_dtypes.bfloat16

B, S, D = 2, 2048, 1024
NH, DK = 16, 64
G = 4              # head groups (cores per batch)
HPG = NH // G      # heads per group = 4
NG = HPG * DK      # dims per group = 256
CH = 512           # q-chunk width
NCH = S // CH      # 4 chunks
NKT = S // 128     # 16 k-tiles
KD = D // 128      # 8 contraction tiles for projections
SCALE = 1.0 / np.sqrt(DK)
VW = 66            # V block stride: 64 dims + ones col + pad (even stride
                   # keeps the DMA-written ones col word-aligned vs the
                   # DVE-written dims -- avoids cross-engine bf16 RMW races)
NWARM = 13         # PE warm-up matmuls bridging the startup DMA window
                   # (first x/w data lands ~11-16us in depending on device
                   # state; a cold PE runs these at ~1GHz, a warm one at 2.4)

TRACE = False          # test harness can set kernel.TRACE = True
LAST_RESULTS = None    # test harness reads kernel.LAST_RESULTS

_NC_CACHE = {}


DEBUG_DUMP = False


def _build_nc(with_biases):
    nc = bacc.Bacc()
    # x inputs and weights arrive pre-tiled in SBUF layout (host rearranges)
    xqT = nc.declare_dram_parameter("xqT", [NCH, 128, KD * CH], BF16, isOutput=False)
    xkT = nc.declare_dram_parameter("xkT", [NCH, 128, KD * CH], BF16, isOutput=False)
    xvT = nc.declare_dram_parameter("xvT", [NCH, 128, KD * CH], BF16, isOutput=False)
    wq = nc.declare_dram_parameter("wq", [128, KD * NG], BF16, isOutput=False)
    wk = nc.declare_dram_parameter("wk", [128, KD * NG], BF16, isOutput=False)
    wv = nc.declare_dram_parameter("wv", [128, KD * NG], BF16, isOutput=False)
    wo = nc.declare_dram_parameter("wo", [128, 2 * D], BF16, isOutput=False)
    mstrip = nc.declare_dram_parameter("mstrip", [128, CH], BF16, isOutput=False)
    ident = nc.declare_dram_parameter("ident", [128, 128], BF16, isOutput=False)
    if with_biases:
        bq = nc.declare_dram_parameter("bq", [128, 2], FP32, isOutput=False)
        bk = nc.declare_dram_parameter("bk", [128, 2], FP32, isOutput=False)
        bv = nc.declare_dram_parameter("bv", [128, 2], FP32, isOutput=False)
    out = nc.declare_dram_parameter("out", [S, D], BF16, isOutput=True)
    if DEBUG_DUMP:
        dbg = {
            nm: nc.declare_dram_parameter(nm, [128, 2 * S], BF16, isOutput=True)
            for nm in ("qd", "kd", "cd")
        }
        dbg["vd"] = nc.declare_dram_parameter("vd", [128, NKT * HPG * VW], BF16,
                                              isOutput=True)

    with tile.TileContext(nc) as tc:
        with (
            tc.tile_pool(name="wpool", bufs=1) as wpool,
            tc.tile_pool(name="cpool", bufs=1) as cpool,
            tc.tile_pool(name="big", bufs=1) as big,
            tc.tile_pool(name="xq", bufs=2) as xqp,
            tc.tile_pool(name="xk", bufs=2) as xkp,
            tc.tile_pool(name="xv", bufs=2) as xvp,
            tc.tile_pool(name="pp", bufs=6) as ppool,
            tc.tile_pool(name="ctp", bufs=2) as ctp,
            tc.tile_pool(name="rbp", bufs=2) as rbp,
            tc.tile_pool(name="ost", bufs=3) as ostp,
            tc.tile_pool(name="ps_x", bufs=2, space="PSUM") as ps_x,
            tc.tile_pool(name="ps_s", bufs=2, space="PSUM") as ps_s,
            tc.tile_pool(name="ps_av", bufs=1, space="PSUM") as ps_av,
        ):
            # ---- resident weights / constants (DMAs emitted lazily below so
            #      Q/K projections can start as early as possible) ----
            wq_sb = wpool.tile([128, KD * NG], BF16, tag="wq")
            wk_sb = wpool.tile([128, KD * NG], BF16, tag="wk")
            wv_sb = wpool.tile([128, KD * NG], BF16, tag="wv")
            wo_sb = wpool.tile([128, 2 * D], BF16, tag="wo")

            def dma_w(w_sb, w_dram, halves=1):
                n = w_sb.shape[1]
                hw = n // halves
                for h in range(halves):
                    nc.sync.dma_start(out=w_sb[:, h * hw:(h + 1) * hw],
                                      in_=w_dram[:, h * hw:(h + 1) * hw])

            mask_sb = cpool.tile([128, CH], BF16, tag="mask")
            ident_sb = cpool.tile([128, 128], BF16, tag="ident")
            wsrc = cpool.tile([128, CH], BF16, tag="wsrc")
            if with_biases:
                bq_sb = cpool.tile([128, 2], FP32, tag="bq")
                bk_sb = cpool.tile([128, 2], FP32, tag="bk")
                bv_sb = cpool.tile([128, 2], FP32, tag="bv")

            # ---- persistent activations ----
            q_sb = [big.tile([128, S], BF16, tag=f"q{m}", name=f"q{m}") for m in range(2)]
            k_sb = [big.tile([128, S], BF16, tag=f"k{m}", name=f"k{m}") for m in range(2)]
            ctx_sb = [big.tile([128, S], BF16, tag=f"ctx{m}", name=f"ctx{m}") for m in range(2)]
            # V: 16 s-tiles x 4 heads x (64 dims + ones col)
            v_sb = big.tile([128, NKT * HPG * VW], BF16, tag="v")

            xt = {}

            def load_x(c, names="qkv", halves=1):
                for nm, pool_, dram_ in (("q", xqp, xqT), ("k", xkp, xkT),
                                         ("v", xvp, xvT)):
                    if nm not in names:
                        continue
                    t_ = pool_.tile([128, KD * CH], BF16, tag="x",
                                    name=f"x{nm}")
                    hw = KD * CH // halves
                    for h in range(halves):
                        nc.sync.dma_start(out=t_[:, h * hw:(h + 1) * hw],
                                          in_=dram_[c, :, h * hw:(h + 1) * hw])
                    xt[(nm, c)] = t_

            def qk_group(c, which):
                # one 128-dim output block of Q^T or K^T for chunk c
                proj, m = divmod(which, 2)
                x_t = xt[("q" if proj == 0 else "k", c)]
                w_sb_ = wq_sb if proj == 0 else wk_sb
                dst = (q_sb if proj == 0 else k_sb)[m]
                pt = ps_x.tile([128, CH], FP32, tag="px", name="pt")
                for kd in range(KD):
                    nc.tensor.matmul(
                        pt[:],
                        lhsT=w_sb_[:, kd * NG + m * 128: kd * NG + m * 128 + 128],
                        rhs=x_t[:, kd * CH: kd * CH + CH],
                        start=(kd == 0), stop=(kd == KD - 1),
                    )
                dstv = dst[:, c * CH:(c + 1) * CH]
                if with_biases:
                    b_sb_ = bq_sb if proj == 0 else bk_sb
                    nc.vector.tensor_scalar_add(dstv, pt[:], b_sb_[:, m:m + 1])
                else:
                    nc.vector.tensor_copy(dstv, pt[:])

            def v_group(c, half):
                # two 128-row s-subtiles of V for chunk c
                x_t = xt[("v", c)]
                pv = ps_x.tile([128, CH], FP32, tag="px", name="pv")
                for ss in (2 * half, 2 * half + 1):
                    col = (ss - 2 * half) * NG
                    for kd in range(KD):
                        nc.tensor.matmul(
                            pv[:, col:col + NG],
                            lhsT=x_t[:, kd * CH + ss * 128:
                                     kd * CH + ss * 128 + 128],
                            rhs=wv_sb[:, kd * NG: kd * NG + NG],
                            start=(kd == 0), stop=(kd == KD - 1),
                        )
                for ss in (2 * half, 2 * half + 1):
                    st = 4 * c + ss
                    col = (ss - 2 * half) * NG
                    dst = v_sb[:, st * HPG * VW: (st + 1) * HPG * VW]
                    dst = dst.rearrange("p (h e) -> p h e", h=HPG)[:, :, 0:64]
                    src = pv[:, col:col + NG].rearrange("p (h e) -> p h e", h=HPG)
                    nc.vector.tensor_copy(dst, src)

            ot_big = {}
            ot_done = {}

            def oproj_group(c, st):
                # partial output projection of one 128-row q block of chunk c;
                # all 4 blocks share one staging tile, DMA'd once per chunk
                r0 = c * CH + st * 128
                if c not in ot_big:
                    ot_big[c] = ostp.tile([128, 4 * D], BF16, tag="ot", name="ot")
                    ot_done[c] = 0
                ot = ot_big[c]
                last = c == NCH - 1
                for mo in range(2):
                    pot = ps_x.tile([128, CH], FP32, tag="px", name="pot")
                    for kk in range(2):
                        nc.tensor.matmul(
                            pot[:],
                            lhsT=ctx_sb[kk][:, r0:r0 + 128],
                            rhs=wo_sb[:, kk * D + mo * CH: kk * D + mo * CH + CH],
                            start=(kk == 0), stop=(kk == 1),
                        )
                    dstc = ot[:, st * D + mo * CH: st * D + mo * CH + CH]
                    if last and mo == 1:
                        # ACT is exp-idle by the last chunk's output stage
                        nc.scalar.copy(dstc, pot[:])
                    else:
                        nc.vector.tensor_copy(dstc, pot[:])
                    if last:
                        # stream each half out as soon as it is staged so the
                        # final DMA (and the exit sequence) starts earlier
                        nc.sync.dma_start(
                            out=out[r0:r0 + 128, mo * CH:(mo + 1) * CH],
                            in_=dstc)
                ot_done[c] += 1
                if last:
                    pass
                elif ot_done[c] == 4:
                    nc.sync.dma_start(
                        out=out[c * CH:(c + 1) * CH, :]
                            .rearrange("(i p) m -> p i m", p=128),
                        in_=ot[:].rearrange("p (i m) -> p i m", i=4),
                    )

            def score_phase(c, hp, kt):
                # scores [k=128, q=w] for the head pair, exp on ACT, and the
                # causal mask-mul for diagonal tiles.  Returns the exp tile;
                # its AV fan-out is issued one kt later (software pipeline)
                # so the in-order PE queue never waits on exp.
                j = kt - 4 * c
                mt = hp // 2
                w = CH - 128 * j if j > 0 else CH
                off = CH - w
                sp = ps_s.tile([128, 2 * CH], FP32, tag="sp", name="sp")
                for i in range(2):
                    po = i * 64
                    nc.tensor.matmul(
                        sp[:, i * CH: i * CH + w],
                        lhsT=k_sb[mt][po:po + 64, kt * 128: kt * 128 + 128],
                        rhs=q_sb[mt][po:po + 64, c * CH + off: (c + 1) * CH],
                        start=True, stop=True,
                    )
                pp = ppool.tile([128, 2 * CH], BF16, tag="p", name="pp")
                if w == CH:
                    # contiguous full-width tile: flat AP is cheaper on ACT
                    nc.scalar.activation(pp[:], sp[:], AF.Exp, scale=SCALE)
                else:
                    pview = pp[:].rearrange("p (t x) -> p t x", t=2)[:, :, 0:w]
                    sview = sp[:].rearrange("p (t x) -> p t x", t=2)[:, :, 0:w]
                    nc.scalar.activation(pview, sview, AF.Exp, scale=SCALE)
                return pp

            def mask_mul(pp):
                mview = pp[:].rearrange("p (t x) -> p t x", t=2)[:, :, 0:128]
                nc.vector.tensor_mul(
                    mview, mview,
                    mask_sb[:, None, 0:128].to_broadcast((128, 2, 128)),
                )

            def att_kt(c, hp, kt, pav, started):
                # v7 issue order: scores+exp, then the AV fan for q-tiles
                # strictly past the diagonal (they depend only on exp), then
                # the mask-mul, then the diagonal q-tile's AVs
                j = kt - 4 * c
                pp = score_phase(c, hp, kt)
                if j >= 0:
                    av_phase(c, hp, kt, pav, pp, started, qlo=max(j, 0) + 1)
                    mask_mul(pp)
                    av_phase(c, hp, kt, pav, pp, started, qhi=max(j, 0) + 1)
                else:
                    av_phase(c, hp, kt, pav, pp, started)

            def av_phase(c, hp, kt, pav, pp, started, qlo=0, qhi=4):
                # Swapped AV fan-out: per q-tile ql, out[q,65] = pp^T @ [V|1].
                # The 4 ql blocks are concurrent accumulation groups at
                # disjoint 65-col regions of one bank.  start=True marks the
                # WHOLE bank pending-zero, so only the first kt=0 write per
                # bank sets it; the other kt=0 writes land on pending-zero
                # bytes and overwrite (= implicit start).  skip_group_check:
                # the sim's per-bank one-pending-group bookkeeping can't
                # represent this (stop at kt=4c+ql clears the whole bank's
                # started flag).
                j = kt - 4 * c
                off = 128 * j if j > 0 else 0
                for i in range(2):
                    for ql in range(max(qlo, max(j, 0)), min(qhi, 4)):
                        vcol = (kt * HPG + hp + i) * VW
                        st = kt == 0 and not started[i]
                        started[i] = True
                        nc.tensor.matmul(
                            pav[i][:, ql * 65: ql * 65 + 65],
                            lhsT=pp[:, i * CH + ql * 128 - off:
                                    i * CH + ql * 128 - off + 128],
                            rhs=v_sb[:, vcol:vcol + 65],
                            start=st, stop=(kt == 4 * c + ql),
                            skip_group_check=True,
                        )

            def norm_part1(c, hp, pav, qls):
                # DVE only: reciprocal of the denominator column (a
                # per-partition scalar) and the normalize-mul into a [q, dims]
                # bf16 staging tile.  No PE instructions, so the PE queue
                # flows straight into the next head-pair's scores.
                nq = len(qls)
                q0 = qls[0]
                rb = rbp.tile([128, 2, 4], FP32, tag="rb", name="rb")
                ctxT = ctp.tile([128, CH], BF16, tag="ctxT", name="ctxT")
                for i in range(2):
                    pav_v = pav[i][:].rearrange("p (q e) -> p q e", e=65)
                    nc.vector.reciprocal_approx_fast(
                        out=rb[:, i, q0:q0 + nq, None],
                        in_=pav_v[:, q0:q0 + nq, 64:65])
                for i in range(2):
                    pav_v = pav[i][:].rearrange("p (q e) -> p q e", e=65)
                    dst = ctxT[:].rearrange("p (q i e) -> p q i e", i=2, e=64)
                    nc.vector.tensor_mul(
                        dst[:, q0:q0 + nq, i, :],
                        pav_v[:, q0:q0 + nq, 0:64],
                        rb[:, i, q0:q0 + nq, None].to_broadcast((128, nq, 64)),
                    )
                return ctxT

            def norm_part2(c, hp, ctxT, qls):
                # PE transpose back to [dims, q] + drain into ctx_sb.  The
                # transposes wait on part1's DVE muls, so this is deferred
                # into the NEXT kt loop (the in-order PE queue would
                # otherwise stall the next head-pair's scores behind them).
                mt = hp // 2
                nq = len(qls)
                q0 = qls[0]
                pt = ps_x.tile([128, 2 * CH], BF16, tag="px", name="ptr")
                for ql in qls:
                    nc.tensor.transpose(pt[:, ql * 128:(ql + 1) * 128],
                                        ctxT[:, ql * 128:(ql + 1) * 128],
                                        ident_sb[:])
                dst = ctx_sb[mt][:, c * CH + q0 * 128: c * CH + (q0 + nq) * 128]
                src = pt[:, q0 * 128:(q0 + nq) * 128]
                if with_biases:
                    nc.vector.tensor_scalar_add(dst, src, bv_sb[:, mt:mt + 1])
                else:
                    nc.vector.tensor_copy(dst, src)

            # ---- schedule ----
            # Warm-up: matmuls on a zeroed scratch tile ramp the PE clock
            # while the startup DMAs stream in.
            vview = v_sb[:].rearrange("p (t e) -> p t e", e=VW)[:, :, 64:65]
            nc.gpsimd.memset(vview, 1.0)
            if DEBUG_DUMP:
                # the VW pad col is never written; init it so the sim's
                # uninitialized-read check passes on the full-v_sb dump
                nc.gpsimd.memset(
                    v_sb[:].rearrange("p (t e) -> p t e", e=VW)[:, :, 65:66], 0.0)
            nc.gpsimd.memset(wsrc[:], 0.0)
            for _ in range(NWARM):
                pw = ps_x.tile([128, CH], FP32, tag="px", name="pw")
                nc.tensor.matmul(pw[:], lhsT=wsrc[:, 0:128], rhs=wsrc[:],
                                 start=True, stop=True)
            # Startup: two HWDGE rings (SP + ACT) stream the startup set in
            # parallel; the ACT ring is idle until the first exp (~16us).
            # Ring order keeps wq + both xq halves landing first so the
            # Q-projections start right as the warm-up runs out.
            # All bulk transfers ride the sync ring (the ACT ring measured
            # ~3x slower); only the tiny mask/ident constants go on the ACT
            # ring so they don't displace the critical startup stream.
            # A 4-byte dummy absorbs any one-time ring-init latency first.
            nc.sync.dma_start(out=ident_sb[0:1, 0:2], in_=ident[0:1, 0:2])
            dma_w(wq_sb, wq, halves=2)
            load_x(0, "q", halves=4)
            nc.scalar.dma_start(out=mask_sb[:], in_=mstrip[:])
            nc.scalar.dma_start(out=ident_sb[:], in_=ident[:])
            if with_biases:
                nc.sync.dma_start(out=bq_sb[:], in_=bq[:])
                nc.sync.dma_start(out=bk_sb[:], in_=bk[:])
                nc.sync.dma_start(out=bv_sb[:], in_=bv[:])
            nc.scalar.dma_start(out=wv_sb[:], in_=wv[:])
            qk_group(0, 0)
            qk_group(0, 1)
            dma_w(wk_sb, wk)
            load_x(0, "k", halves=2)
            qk_group(0, 2)
            qk_group(0, 3)
            load_x(0, "v", halves=2)
            for half in range(2):
                v_group(0, half)
            dma_w(wo_sb, wo)

            # Chunk loop. Work that later chunks depend on is deferred as far
            # as its dependencies allow, so the bigger (attention-heavy, exp-
            # bound) chunks get more PE filler between their matmuls:
            #   - Q(c+1) is the only projection needed before chunk c+1 starts
            #   - K(c)/V(c) are only needed by chunk c's last 4 kt tiles
            #     (the diagonal), so they are emitted inside chunk c itself
            #   - oproj(c-1) fills chunk c; two oproj(1) groups are carried
            #     into chunk 3 (the most exp-bound kt loop)
            #   - each norm's part2 (PE transposes) is deferred to iteration 1
            #     of the NEXT kt loop so it never stalls the in-order PE queue
            carry = []
            part2_q = []
            for c in range(NCH):
                if c < NCH - 1:
                    load_x(c + 1)
                niters = 2 * (4 * c + 4)
                early = []
                if c >= 1:
                    if c >= 2:
                        early += [lambda w=w_: qk_group(c, w) for w_ in (2, 3)]
                    early += [lambda h=h_: v_group(c, h) for h_ in range(2)]
                late = []
                if c < NCH - 1:
                    late += [lambda w=w_: qk_group(c + 1, w) for w_ in (0, 1)]
                if c >= 1:
                    ops = [lambda s=s_, cc=c - 1: oproj_group(cc, s)
                           for s_ in range(4)]
                    if c == NCH - 1:
                        late += ops + carry
                    elif c == NCH - 2:
                        # defer two oproj groups an extra chunk: the last
                        # chunk has the deepest exp deficit
                        late += ops[0:2]
                        carry = ops[2:]
                    else:
                        late += ops
                else:
                    late += [lambda w=w_: qk_group(1, w) for w_ in (2, 3)]
                sched = [[] for _ in range(niters)]
                # early groups: finish before iteration 4c of the hp=0 loop
                span_e = max(1, 4 * c - 1)
                for gi, g in enumerate(early):
                    pos = (gi * span_e) // (len(early) - 1) if len(early) > 1 else 0
                    sched[min(span_e, pos)].append(g)
                # pin the first two late groups at the hp0->hp2 and
                # hp2->next-chunk boundary iterations: the next loop's first
                # scores wait there for the old hp's last exps to free their
                # psum tiles, so the PE needs filler
                # (for the last chunk keep the final iterations clear -- the
                # per-q-tile tail chain owns them)
                bpos = [4 * c + 3, niters - 1 if c < NCH - 1 else niters - 6]
                for gi, g in enumerate(late):
                    if gi < 2 and c >= 1:
                        pos = bpos[gi]
                    else:
                        pos = min(niters - 1,
                                  (gi + 1) * niters // (len(late) + 1))
                    sched[pos].append(g)
                it = 0
                last_c = c == NCH - 1
                for hp in (0, 2):
                    pav = [ps_av.tile([128, 260], FP32, tag=f"av{i}",
                                      name=f"pav{i}")
                           for i in range(2)]
                    started = [False, False]
                    pend = None
                    nkt = 4 * c + 4
                    for kt in range(nkt):
                        att_kt(c, hp, kt, pav, started)
                        if kt == 1:
                            for g in part2_q:
                                g()
                            part2_q = []
                        if pend is not None:
                            # tail pipeline: transpose + oproj of the previous
                            # q-tile, one iteration after its norm part1
                            ctxT_, ql_ = pend
                            norm_part2(c, hp, ctxT_, [ql_])
                            oproj_group(c, ql_)
                            pend = None
                        for g in sched[it]:
                            g()
                        it += 1
                        if last_c and hp == 2 and kt >= 4 * c:
                            # per-q-tile tail: q-tile ql's AV accumulation
                            # stopped at kt == 4c+ql; normalize it now
                            ql = kt - 4 * c
                            pend = (norm_part1(c, hp, pav, [ql]), ql)
                    if last_c and hp == 2:
                        if pend is not None:
                            ctxT_, ql_ = pend
                            norm_part2(c, hp, ctxT_, [ql_])
                            oproj_group(c, ql_)
                    else:
                        ctxT = norm_part1(c, hp, pav, [0, 1, 2, 3])
                        if with_biases:
                            part2_q.append(
                                lambda cc=c, hh=hp, ct=ctxT: norm_part2(
                                    cc, hh, ct, [0, 1, 2, 3]))
                        else:
                            # XBAR-transposing DMA does [q,dims]->[dims,q]
                            # per 128-slab entirely off the PE/DVE; the
                            # consumer (oproj) runs a chunk later, hiding
                            # the DMA latency
                            mt = hp // 2
                            nc.sync.dma_start_transpose(
                                out=ctx_sb[mt][:, c * CH:(c + 1) * CH]
                                    .rearrange("p (i j) -> p i j", j=128),
                                in_=ctxT[:])

            if DEBUG_DUMP:
                for nm, pair in (("qd", q_sb), ("kd", k_sb), ("cd", ctx_sb)):
                    for m in range(2):
                        nc.sync.dma_start(out=dbg[nm][:, m * S:(m + 1) * S],
                                          in_=pair[m][:])
                nc.sync.dma_start(out=dbg["vd"][:], in_=v_sb[:])

    nc.compile()
    return nc


def _get_nc(with_biases):
    if with_biases not in _NC_CACHE:
        _NC_CACHE[with_biases] = _build_nc(with_biases)
    return _NC_CACHE[with_biases]


def _pretile_x(x):
    # [S, D] activation -> [NCH, 128, KD*CH] bf16 in SBUF layout:
    # out[c, p, kd*CH + s] = x[c*CH + s, kd*128 + p]
    xt = np.ascontiguousarray(x.T)                       # [D, S]
    t = xt.reshape(KD, 128, NCH, CH).transpose(2, 1, 0, 3)
    return np.ascontiguousarray(t.reshape(NCH, 128, KD * CH)).astype(BF16_NP)


def _pretile_w(w):
    # [D, n] weight -> [128, KD*n] bf16: out[p, kd*n + j] = w[kd*128 + p, j]
    n = w.shape[1]
    t = w.reshape(KD if w.shape[0] == D else 2, 128, n).transpose(1, 0, 2)
    return np.ascontiguousarray(t.reshape(128, -1)).astype(BF16_NP)


def _mask_strip():
    # strip[p, y] = 1.0 iff y >= p; with the causal sub-range offset applied
    # to the q-columns, every diagonal k-tile masks with strip[:, 0:w]
    y = np.arange(CH)[None, :]
    p = np.arange(128)[:, None]
    return (y >= p).astype(BF16_NP)


def _reference_fallback(query, key, value, mask, wq, bq, wk, bk, wv, bv, wo, bo):
    out = np.empty((B, S, D), np.float32)
    for b in range(B):
        Q = (query[b] @ wq + bq).reshape(S, NH, DK).transpose(1, 0, 2)
        K = (key[b] @ wk + bk).reshape(S, NH, DK).transpose(1, 0, 2)
        V = (value[b] @ wv + bv).reshape(S, NH, DK).transpose(1, 0, 2)
        sc = np.einsum("hqd,hkd->hqk", Q, K).astype(np.float32) / np.sqrt(DK)
        sc = np.where(mask[b][None] == 0, -1.0e9, sc)
        sc -= sc.max(-1, keepdims=True)
        e = np.exp(sc)
        attn = e / e.sum(-1, keepdims=True)
        ctx = np.einsum("hqk,hkd->hqd", attn, V).transpose(1, 0, 2).reshape(S, D)
        out[b] = ctx @ wo + bo
    return out


def kernel(query, key, value, mask, wq, bq, wk, bk, wv, bv, wo, bo):
    global LAST_RESULTS
    query = np.asarray(query, np.float32)
    key = np.asarray(key, np.float32)
    value = np.asarray(value, np.float32)
    mask = np.asarray(mask)
    wq, bq = np.asarray(wq, np.float32), np.asarray(bq, np.float32)
    wk, bk = np.asarray(wk, np.float32), np.asarray(bk, np.float32)
    wv, bv = np.asarray(wv, np.float32), np.asarray(bv, np.float32)
    wo, bo = np.asarray(wo, np.float32), np.asarray(bo, np.float32)

    tril = np.tril(np.ones((S, S), mask.dtype))
    if not all(np.array_equal(mask[b], tril) for b in range(B)):
        return _reference_fallback(query, key, value, mask, wq, bq, wk, bk,
                                   wv, bv, wo, bo)

    with_biases = any(np.any(b_ != 0) for b_ in (bq, bk, bv))

    strip = _mask_strip()
    xT = {}
    for b in range(B):
        xT[("q", b)] = _pretile_x(query[b])
        xT[("k", b)] = _pretile_x(key[b])
        xT[("v", b)] = _pretile_x(value[b])

    in_maps = []
    for core in range(8):
        b, g = core // G, core % G
        cs = slice(g * NG, (g + 1) * NG)
        m = {
            "xqT": xT[("q", b)],
            "xkT": xT[("k", b)],
            "xvT": xT[("v", b)],
            "wq": _pretile_w(wq[:, cs]),
            "wk": _pretile_w(wk[:, cs]),
            "wv": _pretile_w(wv[:, cs]),
            "wo": _pretile_w(wo[cs, :]),
            "mstrip": strip,
            "ident": np.eye(128, dtype=BF16_NP),
        }
        if with_biases:
            m["bq"] = np.ascontiguousarray(bq[cs].reshape(2, 128).T)
            m["bk"] = np.ascontiguousarray(bk[cs].reshape(2, 128).T)
            m["bv"] = np.ascontiguousarray(bv[cs].reshape(2, 128).T)
        in_maps.append(m)

    nc = _get_nc(with_biases)
    res = run_bass_kernel_spmd(nc, in_maps, list(range(8)), trace=TRACE)
    LAST_RESULTS = res

    out = np.empty((B, S, D), np.float32)
    for b in range(B):
        acc = res.results[b * G]["out"].astype(np.float32)
        for g in range(1, G):
            acc = acc + res.results[b * G + g]["out"].astype(np.float32)
        out[b] = acc + bo
    return out


# revision 31
# speedup vs baseline: 1.1021x; 1.1021x over previous
"""Multi-head attention (B=2, S=2048, D=1024, H=16 heads, causal) on 8 TRN2 cores.

Sharding: core i handles batch b=i//4 and head group g=i%4 (4 heads = 256 dims).
Each core computes QKV projections for its head group, causal attention, and a
partial output projection (its 256-dim slice of the contraction). Host sums the
4 partials per batch and adds the output bias.

v3 design (vs v2 at ~168us):
  The HW trace shows PE matmuls never overlap on the array -- cost is simply
  sum(output free size) cycles over all matmuls.  v2's AV matmuls
  (out [65, w] = V|1^T x pp) streamed N=w for each head: 69632 cycles, plus a
  16384-cycle ones-matmul chain to broadcast softmax denominators.  v3 swaps
  the AV operands: per 128-wide q-tile, out[q,65] = pp_tile^T @ [V|ones]
  streams only N=65 per (head, q-tile, k-tile): 35360 cycles, 100% array
  utilization, and the denominator lands in column 64 as a per-PARTITION
  scalar, so normalization is a cheap tensor mul with a [P,1] reciprocal
  (no PE broadcast, no [1,512] single-lane copies).  ctx then comes out
  [q, dims] and a bf16 PE transpose (1 cyc/row, 4096 cycles total) restores
  the [dims, q] layout the output projection needs.
  Other changes:
  - PE warm-up matmuls on a zeroed scratch tile fill the startup DMA window
    so the PE p-state/HAM clock is at full speed when real work arrives
  - startup DMAs use fewer trigger instructions (each costs ~600ns on the
    sync queue) ordered wq, xq, wk, xk, ...
  - last chunk runs norm/transpose/oproj per q-tile as soon as that q-tile's
    AV accumulation stops, shrinking the serial tail
  - two oproj groups of chunk 1 are deferred to chunk 3, whose kt loop is the
    most exp(ACT)-bound, to keep the PE fed there
"""
import sys

import numpy as np

try:
    import concourse.bass as bass  # noqa: F401
except ImportError:
    sys.path.insert(0, "/opt/trn_rl_repo")

import ml_dtypes

import concourse.bass as bass  # noqa: F401
import concourse.mybir as mybir
import concourse.tile as tile
from concourse import bacc
from concourse.bass_utils import run_bass_kernel_spmd

FP32 = mybir.dt.float32
F32R = mybir.dt.float32r
BF16 = mybir.dt.bfloat16
AF = mybir.ActivationFunctionType
BF16_NP = ml_dtypes.bfloat16

B, S, D = 2, 2048, 1024
NH, DK = 16, 64
G = 4              # head groups (cores per batch)
HPG = NH // G      # heads per group = 4
NG = HPG * DK      # dims per group = 256
CH = 512           # q-chunk width
NCH = S // CH      # 4 chunks
NKT = S // 128     # 16 k-tiles
KD = D // 128      # 8 contraction tiles for projections
SCALE = 1.0 / np.sqrt(DK)
VW = 66            # V block stride: 64 dims + ones col + pad (even stride
                   # keeps the DMA-written ones col word-aligned vs the
                   # DVE-written dims -- avoids cross-engine bf16 RMW races)
NWARM = 13         # PE warm-up matmuls bridging the startup DMA window
                   # (first x/w data lands ~11-16us in depending on device
                   # state; a cold PE runs these at ~1GHz, a warm one at 2.4)

TRACE = False          # test harness can set kernel.TRACE = True
LAST_RESULTS = None    # test harness reads kernel.LAST_RESULTS

_NC_CACHE = {}


DEBUG_DUMP = False


def _build_nc(with_biases):
    nc = bacc.Bacc()
    # x inputs and weights arrive pre-tiled in SBUF layout (host rearranges)
    xqT = nc.declare_dram_parameter("xqT", [NCH, 128, KD * CH], BF16, isOutput=False)
    xkT = nc.declare_dram_parameter("xkT", [NCH, 128, KD * CH], BF16, isOutput=False)
    xvT = nc.declare_dram_parameter("xvT", [NCH, 128, KD * CH], BF16, isOutput=False)
    wq = nc.declare_dram_parameter("wq", [128, KD * NG], BF16, isOutput=False)
    wk = nc.declare_dram_parameter("wk", [128, KD * NG], BF16, isOutput=False)
    wv = nc.declare_dram_parameter("wv", [128, KD * NG], BF16, isOutput=False)
    wo = nc.declare_dram_parameter("wo", [128, 2 * D], BF16, isOutput=False)
    mstrip = nc.declare_dram_parameter("mstrip", [128, CH], BF16, isOutput=False)
    ident = nc.declare_dram_parameter("ident", [128, 128], BF16, isOutput=False)
    if with_biases:
        bq = nc.declare_dram_parameter("bq", [128, 2], FP32, isOutput=False)
        bk = nc.declare_dram_parameter("bk", [128, 2], FP32, isOutput=False)
        bv = nc.declare_dram_parameter("bv", [128, 2], FP32, isOutput=False)
    out = nc.declare_dram_parameter("out", [S, D], BF16, isOutput=True)
    if DEBUG_DUMP:
        dbg = {
            nm: nc.declare_dram_parameter(nm, [128, 2 * S], BF16, isOutput=True)
            for nm in ("qd", "kd", "cd")
        }
        dbg["vd"] = nc.declare_dram_parameter("vd", [128, NKT * HPG * VW], BF16,
                                              isOutput=True)

    with tile.TileContext(nc) as tc:
        with (
            tc.tile_pool(name="wpool", bufs=1) as wpool,
            tc.tile_pool(name="cpool", bufs=1) as cpool,
            tc.tile_pool(name="big", bufs=1) as big,
            tc.tile_pool(name="xq", bufs=2) as xqp,
            tc.tile_pool(name="xk", bufs=2) as xkp,
            tc.tile_pool(name="xv", bufs=2) as xvp,
            tc.tile_pool(name="pp", bufs=6) as ppool,
            tc.tile_pool(name="ctp", bufs=2) as ctp,
            tc.tile_pool(name="rbp", bufs=2) as rbp,
            tc.tile_pool(name="ost", bufs=3) as ostp,
            tc.tile_pool(name="ps_x", bufs=2, space="PSUM") as ps_x,
            tc.tile_pool(name="ps_s", bufs=2, space="PSUM") as ps_s,
            tc.tile_pool(name="ps_av", bufs=1, space="PSUM") as ps_av,
        ):
            # ---- resident weights / constants (DMAs emitted lazily below so
            #      Q/K projections can start as early as possible) ----
            wq_sb = wpool.tile([128, KD * NG], BF16, tag="wq")
            wk_sb = wpool.tile([128, KD * NG], BF16, tag="wk")
            wv_sb = wpool.tile([128, KD * NG], BF16, tag="wv")
            wo_sb = wpool.tile([128, 2 * D], BF16, tag="wo")

            def dma_w(w_sb, w_dram, halves=1):
                n = w_sb.shape[1]
                hw = n // halves
                for h in range(halves):
                    nc.sync.dma_start(out=w_sb[:, h * hw:(h + 1) * hw],
                                      in_=w_dram[:, h * hw:(h + 1) * hw])

            mask_sb = cpool.tile([128, CH], BF16, tag="mask")
            ident_sb = cpool.tile([128, 128], BF16, tag="ident")
            wsrc = cpool.tile([128, CH], BF16, tag="wsrc")
            if with_biases:
                bq_sb = cpool.tile([128, 2], FP32, tag="bq")
                bk_sb = cpool.tile([128, 2], FP32, tag="bk")
                bv_sb = cpool.tile([128, 2], FP32, tag="bv")

            # ---- persistent activations ----
            q_sb = [big.tile([128, S], BF16, tag=f"q{m}", name=f"q{m}") for m in range(2)]
            k_sb = [big.tile([128, S], BF16, tag=f"k{m}", name=f"k{m}") for m in range(2)]
            ctx_sb = [big.tile([128, S], BF16, tag=f"ctx{m}", name=f"ctx{m}") for m in range(2)]
            # V: 16 s-tiles x 4 heads x (64 dims + ones col)
            v_sb = big.tile([128, NKT * HPG * VW], BF16, tag="v")

            xt = {}

            def load_x(c, names="qkv", halves=1):
                for nm, pool_, dram_ in (("q", xqp, xqT), ("k", xkp, xkT),
                                         ("v", xvp, xvT)):
                    if nm not in names:
                        continue
                    t_ = pool_.tile([128, KD * CH], BF16, tag="x",
                                    name=f"x{nm}")
                    hw = KD * CH // halves
                    for h in range(halves):
                        nc.sync.dma_start(out=t_[:, h * hw:(h + 1) * hw],
                                          in_=dram_[c, :, h * hw:(h + 1) * hw])
                    xt[(nm, c)] = t_

            def qk_group(c, which):
                # one 128-dim output block of Q^T or K^T for chunk c
                proj, m = divmod(which, 2)
                x_t = xt[("q" if proj == 0 else "k", c)]
                w_sb_ = wq_sb if proj == 0 else wk_sb
                dst = (q_sb if proj == 0 else k_sb)[m]
                pt = ps_x.tile([128, CH], FP32, tag="px", name="pt")
                for kd in range(KD):
                    nc.tensor.matmul(
                        pt[:],
                        lhsT=w_sb_[:, kd * NG + m * 128: kd * NG + m * 128 + 128],
                        rhs=x_t[:, kd * CH: kd * CH + CH],
                        start=(kd == 0), stop=(kd == KD - 1),
                    )
                dstv = dst[:, c * CH:(c + 1) * CH]
                if with_biases:
                    b_sb_ = bq_sb if proj == 0 else bk_sb
                    nc.vector.tensor_scalar_add(dstv, pt[:], b_sb_[:, m:m + 1])
                else:
                    nc.vector.tensor_copy(dstv, pt[:])

            def v_group(c, half):
                # two 128-row s-subtiles of V for chunk c
                x_t = xt[("v", c)]
                pv = ps_x.tile([128, CH], FP32, tag="px", name="pv")
                for ss in (2 * half, 2 * half + 1):
                    col = (ss - 2 * half) * NG
                    for kd in range(KD):
                        nc.tensor.matmul(
                            pv[:, col:col + NG],
                            lhsT=x_t[:, kd * CH + ss * 128:
                                     kd * CH + ss * 128 + 128],
                            rhs=wv_sb[:, kd * NG: kd * NG + NG],
                            start=(kd == 0), stop=(kd == KD - 1),
                        )
                for ss in (2 * half, 2 * half + 1):
                    st = 4 * c + ss
                    col = (ss - 2 * half) * NG
                    dst = v_sb[:, st * HPG * VW: (st + 1) * HPG * VW]
                    dst = dst.rearrange("p (h e) -> p h e", h=HPG)[:, :, 0:64]
                    src = pv[:, col:col + NG].rearrange("p (h e) -> p h e", h=HPG)
                    nc.vector.tensor_copy(dst, src)

            ot_big = {}
            ot_done = {}

            def oproj_group(c, st):
                # partial output projection of one 128-row q block of chunk c;
                # all 4 blocks share one staging tile, DMA'd once per chunk
                r0 = c * CH + st * 128
                if c not in ot_big:
                    ot_big[c] = ostp.tile([128, 4 * D], BF16, tag="ot", name="ot")
                    ot_done[c] = 0
                ot = ot_big[c]
                last = c == NCH - 1
                for mo in range(2):
                    pot = ps_x.tile([128, CH], FP32, tag="px", name="pot")
                    for kk in range(2):
                        nc.tensor.matmul(
                            pot[:],
                            lhsT=ctx_sb[kk][:, r0:r0 + 128],
                            rhs=wo_sb[:, kk * D + mo * CH: kk * D + mo * CH + CH],
                            start=(kk == 0), stop=(kk == 1),
                        )
                    dstc = ot[:, st * D + mo * CH: st * D + mo * CH + CH]
                    if last and mo == 1:
                        # ACT is exp-idle by the last chunk's output stage
                        nc.scalar.copy(dstc, pot[:])
                    else:
                        nc.vector.tensor_copy(dstc, pot[:])
                    if last:
                        # stream each half out as soon as it is staged so the
                        # final DMA (and the exit sequence) starts earlier;
                        # the mo=1 half rides the ACT ring (exp-idle by now)
                        # so both halves fly in parallel
                        eng = nc.scalar if mo == 1 else nc.sync
                        eng.dma_start(
                            out=out[r0:r0 + 128, mo * CH:(mo + 1) * CH],
                            in_=dstc)
                ot_done[c] += 1
                if last:
                    pass
                elif ot_done[c] == 4:
                    nc.sync.dma_start(
                        out=out[c * CH:(c + 1) * CH, :]
                            .rearrange("(i p) m -> p i m", p=128),
                        in_=ot[:].rearrange("p (i m) -> p i m", i=4),
                    )

            def score_phase(c, hp, kt):
                # scores [k=128, q=w] for the head pair, exp on ACT, and the
                # causal mask-mul for diagonal tiles.  Returns the exp tile;
                # its AV fan-out is issued one kt later (software pipeline)
                # so the in-order PE queue never waits on exp.
                j = kt - 4 * c
                mt = hp // 2
                w = CH - 128 * j if j > 0 else CH
                off = CH - w
                sp = ps_s.tile([128, 2 * CH], FP32, tag="sp", name="sp")
                for i in range(2):
                    po = i * 64
                    nc.tensor.matmul(
                        sp[:, i * CH: i * CH + w],
                        lhsT=k_sb[mt][po:po + 64, kt * 128: kt * 128 + 128],
                        rhs=q_sb[mt][po:po + 64, c * CH + off: (c + 1) * CH],
                        start=True, stop=True,
                    )
                pp = ppool.tile([128, 2 * CH], BF16, tag="p", name="pp")
                if w == CH:
                    # contiguous full-width tile: flat AP is cheaper on ACT
                    nc.scalar.activation(pp[:], sp[:], AF.Exp, scale=SCALE)
                else:
                    pview = pp[:].rearrange("p (t x) -> p t x", t=2)[:, :, 0:w]
                    sview = sp[:].rearrange("p (t x) -> p t x", t=2)[:, :, 0:w]
                    nc.scalar.activation(pview, sview, AF.Exp, scale=SCALE)
                return pp

            def mask_mul(pp):
                mview = pp[:].rearrange("p (t x) -> p t x", t=2)[:, :, 0:128]
                nc.vector.tensor_mul(
                    mview, mview,
                    mask_sb[:, None, 0:128].to_broadcast((128, 2, 128)),
                )

            def att_kt(c, hp, kt, pav, started):
                # v7 issue order: scores+exp, then the AV fan for q-tiles
                # strictly past the diagonal (they depend only on exp), then
                # the mask-mul, then the diagonal q-tile's AVs
                j = kt - 4 * c
                pp = score_phase(c, hp, kt)
                if j >= 0:
                    av_phase(c, hp, kt, pav, pp, started, qlo=max(j, 0) + 1)
                    mask_mul(pp)
                    av_phase(c, hp, kt, pav, pp, started, qhi=max(j, 0) + 1)
                else:
                    av_phase(c, hp, kt, pav, pp, started)

            def av_phase(c, hp, kt, pav, pp, started, qlo=0, qhi=4):
                # Swapped AV fan-out: per q-tile ql, out[q,65] = pp^T @ [V|1].
                # The 4 ql blocks are concurrent accumulation groups at
                # disjoint 65-col regions of one bank.  start=True marks the
                # WHOLE bank pending-zero, so only the first kt=0 write per
                # bank sets it; the other kt=0 writes land on pending-zero
                # bytes and overwrite (= implicit start).  skip_group_check:
                # the sim's per-bank one-pending-group bookkeeping can't
                # represent this (stop at kt=4c+ql clears the whole bank's
                # started flag).
                j = kt - 4 * c
                off = 128 * j if j > 0 else 0
                for i in range(2):
                    for ql in range(max(qlo, max(j, 0)), min(qhi, 4)):
                        vcol = (kt * HPG + hp + i) * VW
                        st = kt == 0 and not started[i]
                        started[i] = True
                        nc.tensor.matmul(
                            pav[i][:, ql * 65: ql * 65 + 65],
                            lhsT=pp[:, i * CH + ql * 128 - off:
                                    i * CH + ql * 128 - off + 128],
                            rhs=v_sb[:, vcol:vcol + 65],
                            start=st, stop=(kt == 4 * c + ql),
                            skip_group_check=True,
                        )

            def norm_part1(c, hp, pav, qls):
                # DVE only: reciprocal of the denominator column (a
                # per-partition scalar) and the normalize-mul into a [q, dims]
                # bf16 staging tile.  No PE instructions, so the PE queue
                # flows straight into the next head-pair's scores.
                nq = len(qls)
                q0 = qls[0]
                rb = rbp.tile([128, 2, 4], FP32, tag="rb", name="rb")
                ctxT = ctp.tile([128, CH], BF16, tag="ctxT", name="ctxT")
                for i in range(2):
                    pav_v = pav[i][:].rearrange("p (q e) -> p q e", e=65)
                    nc.vector.reciprocal_approx_fast(
                        out=rb[:, i, q0:q0 + nq, None],
                        in_=pav_v[:, q0:q0 + nq, 64:65])
                for i in range(2):
                    pav_v = pav[i][:].rearrange("p (q e) -> p q e", e=65)
                    dst = ctxT[:].rearrange("p (q i e) -> p q i e", i=2, e=64)
                    nc.vector.tensor_mul(
                        dst[:, q0:q0 + nq, i, :],
                        pav_v[:, q0:q0 + nq, 0:64],
                        rb[:, i, q0:q0 + nq, None].to_broadcast((128, nq, 64)),
                    )
                return ctxT

            def norm_part2(c, hp, ctxT, qls):
                # PE transpose back to [dims, q] + drain into ctx_sb.  The
                # transposes wait on part1's DVE muls, so this is deferred
                # into the NEXT kt loop (the in-order PE queue would
                # otherwise stall the next head-pair's scores behind them).
                mt = hp // 2
                nq = len(qls)
                q0 = qls[0]
                pt = ps_x.tile([128, 2 * CH], BF16, tag="px", name="ptr")
                for ql in qls:
                    nc.tensor.transpose(pt[:, ql * 128:(ql + 1) * 128],
                                        ctxT[:, ql * 128:(ql + 1) * 128],
                                        ident_sb[:])
                dst = ctx_sb[mt][:, c * CH + q0 * 128: c * CH + (q0 + nq) * 128]
                src = pt[:, q0 * 128:(q0 + nq) * 128]
                if with_biases:
                    nc.vector.tensor_scalar_add(dst, src, bv_sb[:, mt:mt + 1])
                else:
                    nc.vector.tensor_copy(dst, src)

            # ---- schedule ----
            # Warm-up: matmuls on a zeroed scratch tile ramp the PE clock
            # while the startup DMAs stream in.
            vview = v_sb[:].rearrange("p (t e) -> p t e", e=VW)[:, :, 64:65]
            nc.gpsimd.memset(vview, 1.0)
            if DEBUG_DUMP:
                # the VW pad col is never written; init it so the sim's
                # uninitialized-read check passes on the full-v_sb dump
                nc.gpsimd.memset(
                    v_sb[:].rearrange("p (t e) -> p t e", e=VW)[:, :, 65:66], 0.0)
            nc.gpsimd.memset(wsrc[:], 0.0)
            for _ in range(NWARM):
                pw = ps_x.tile([128, CH], FP32, tag="px", name="pw")
                nc.tensor.matmul(pw[:], lhsT=wsrc[:, 0:128], rhs=wsrc[:],
                                 start=True, stop=True)
            # Startup: two HWDGE rings (SP + ACT) stream the startup set in
            # parallel; the ACT ring is idle until the first exp (~16us).
            # Ring order keeps wq + both xq halves landing first so the
            # Q-projections start right as the warm-up runs out.
            # All bulk transfers ride the sync ring (the ACT ring measured
            # ~3x slower); only the tiny mask/ident constants go on the ACT
            # ring so they don't displace the critical startup stream.
            # A 4-byte dummy absorbs any one-time ring-init latency first.
            nc.sync.dma_start(out=ident_sb[0:1, 0:2], in_=ident[0:1, 0:2])
            dma_w(wq_sb, wq, halves=2)
            load_x(0, "q", halves=4)
            nc.scalar.dma_start(out=mask_sb[:], in_=mstrip[:])
            nc.scalar.dma_start(out=ident_sb[:], in_=ident[:])
            if with_biases:
                nc.sync.dma_start(out=bq_sb[:], in_=bq[:])
                nc.sync.dma_start(out=bk_sb[:], in_=bk[:])
                nc.sync.dma_start(out=bv_sb[:], in_=bv[:])
            nc.scalar.dma_start(out=wv_sb[:], in_=wv[:])
            qk_group(0, 0)
            qk_group(0, 1)
            dma_w(wk_sb, wk)
            load_x(0, "k", halves=2)
            qk_group(0, 2)
            qk_group(0, 3)
            load_x(0, "v", halves=2)
            for half in range(2):
                v_group(0, half)
            dma_w(wo_sb, wo)

            # Chunk loop. Work that later chunks depend on is deferred as far
            # as its dependencies allow, so the bigger (attention-heavy, exp-
            # bound) chunks get more PE filler between their matmuls:
            #   - Q(c+1) is the only projection needed before chunk c+1 starts
            #   - K(c)/V(c) are only needed by chunk c's last 4 kt tiles
            #     (the diagonal), so they are emitted inside chunk c itself
            #   - oproj(c-1) fills chunk c; two oproj(1) groups are carried
            #     into chunk 3 (the most exp-bound kt loop)
            #   - each norm's part2 (PE transposes) is deferred to iteration 1
            #     of the NEXT kt loop so it never stalls the in-order PE queue
            carry = []
            part2_q = []
            for c in range(NCH):
                if c < NCH - 1:
                    load_x(c + 1)
                niters = 2 * (4 * c + 4)
                early = []
                if c >= 1:
                    if c >= 2:
                        early += [lambda w=w_: qk_group(c, w) for w_ in (2, 3)]
                    early += [lambda h=h_: v_group(c, h) for h_ in range(2)]
                late = []
                if c < NCH - 1:
                    late += [lambda w=w_: qk_group(c + 1, w) for w_ in (0, 1)]
                if c >= 1:
                    ops = [lambda s=s_, cc=c - 1: oproj_group(cc, s)
                           for s_ in range(4)]
                    if c == NCH - 1:
                        late += ops + carry
                    elif c == NCH - 2:
                        # defer two oproj groups an extra chunk: the last
                        # chunk has the deepest exp deficit
                        late += ops[0:2]
                        carry = ops[2:]
                    else:
                        late += ops
                else:
                    late += [lambda w=w_: qk_group(1, w) for w_ in (2, 3)]
                sched = [[] for _ in range(niters)]
                # early groups: finish before iteration 4c of the hp=0 loop
                span_e = max(1, 4 * c - 1)
                for gi, g in enumerate(early):
                    pos = (gi * span_e) // (len(early) - 1) if len(early) > 1 else 0
                    sched[min(span_e, pos)].append(g)
                # pin the first two late groups at the hp0->hp2 and
                # hp2->next-chunk boundary iterations: the next loop's first
                # scores wait there for the old hp's last exps to free their
                # psum tiles, so the PE needs filler
                # (for the last chunk keep the final iterations clear -- the
                # per-q-tile tail chain owns them); when groups are plentiful
                # put a second one at the hp0->hp2 boundary (the stall there
                # is ~2 group-lengths deep)
                pins = [4 * c + 3, niters - 1 if c < NCH - 1 else niters - 6]
                if len(late) >= 5:
                    pins.append(4 * c + 3)
                for gi, g in enumerate(late):
                    if gi < len(pins) and c >= 1:
                        pos = pins[gi]
                    else:
                        pos = min(niters - 1,
                                  (gi + 1) * niters // (len(late) + 1))
                    sched[pos].append(g)
                it = 0
                last_c = c == NCH - 1
                for hp in (0, 2):
                    pav = [ps_av.tile([128, 260], FP32, tag=f"av{i}",
                                      name=f"pav{i}")
                           for i in range(2)]
                    started = [False, False]
                    pend = None
                    nkt = 4 * c + 4
                    for kt in range(nkt):
                        att_kt(c, hp, kt, pav, started)
                        if kt == 1:
                            for g in part2_q:
                                g()
                            part2_q = []
                        if pend is not None:
                            # tail pipeline: transpose + oproj of the previous
                            # q-tile, one iteration after its norm part1
                            ctxT_, ql_ = pend
                            norm_part2(c, hp, ctxT_, [ql_])
                            oproj_group(c, ql_)
                            pend = None
                        for g in sched[it]:
                            g()
                        it += 1
                        if last_c and hp == 2 and kt >= 4 * c:
                            # per-q-tile tail: q-tile ql's AV accumulation
                            # stopped at kt == 4c+ql; normalize it now
                            ql = kt - 4 * c
                            pend = (norm_part1(c, hp, pav, [ql]), ql)
                    if last_c and hp == 2:
                        if pend is not None:
                            ctxT_, ql_ = pend
                            norm_part2(c, hp, ctxT_, [ql_])
                            oproj_group(c, ql_)
                    else:
                        ctxT = norm_part1(c, hp, pav, [0, 1, 2, 3])
                        if with_biases:
                            part2_q.append(
                                lambda cc=c, hh=hp, ct=ctxT: norm_part2(
                                    cc, hh, ct, [0, 1, 2, 3]))
                        else:
                            # XBAR-transposing DMA does [q,dims]->[dims,q]
                            # per 128-slab entirely off the PE/DVE; the
                            # consumer (oproj) runs a chunk later, hiding
                            # the DMA latency
                            mt = hp // 2
                            nc.sync.dma_start_transpose(
                                out=ctx_sb[mt][:, c * CH:(c + 1) * CH]
                                    .rearrange("p (i j) -> p i j", j=128),
                                in_=ctxT[:])

            if DEBUG_DUMP:
                for nm, pair in (("qd", q_sb), ("kd", k_sb), ("cd", ctx_sb)):
                    for m in range(2):
                        nc.sync.dma_start(out=dbg[nm][:, m * S:(m + 1) * S],
                                          in_=pair[m][:])
                nc.sync.dma_start(out=dbg["vd"][:], in_=v_sb[:])

    nc.compile()
    return nc


def _get_nc(with_biases):
    if with_biases not in _NC_CACHE:
        _NC_CACHE[with_biases] = _build_nc(with_biases)
    return _NC_CACHE[with_biases]


def _pretile_x(x):
    # [S, D] activation -> [NCH, 128, KD*CH] bf16 in SBUF layout:
    # out[c, p, kd*CH + s] = x[c*CH + s, kd*128 + p]
    xt = np.ascontiguousarray(x.T)                       # [D, S]
    t = xt.reshape(KD, 128, NCH, CH).transpose(2, 1, 0, 3)
    return np.ascontiguousarray(t.reshape(NCH, 128, KD * CH)).astype(BF16_NP)


def _pretile_w(w):
    # [D, n] weight -> [128, KD*n] bf16: out[p, kd*n + j] = w[kd*128 + p, j]
    n = w.shape[1]
    t = w.reshape(KD if w.shape[0] == D else 2, 128, n).transpose(1, 0, 2)
    return np.ascontiguousarray(t.reshape(128, -1)).astype(BF16_NP)


def _mask_strip():
    # strip[p, y] = 1.0 iff y >= p; with the causal sub-range offset applied
    # to the q-columns, every diagonal k-tile masks with strip[:, 0:w]
    y = np.arange(CH)[None, :]
    p = np.arange(128)[:, None]
    return (y >= p).astype(BF16_NP)


def _reference_fallback(query, key, value, mask, wq, bq, wk, bk, wv, bv, wo, bo):
    out = np.empty((B, S, D), np.float32)
    for b in range(B):
        Q = (query[b] @ wq + bq).reshape(S, NH, DK).transpose(1, 0, 2)
        K = (key[b] @ wk + bk).reshape(S, NH, DK).transpose(1, 0, 2)
        V = (value[b] @ wv + bv).reshape(S, NH, DK).transpose(1, 0, 2)
        sc = np.einsum("hqd,hkd->hqk", Q, K).astype(np.float32) / np.sqrt(DK)
        sc = np.where(mask[b][None] == 0, -1.0e9, sc)
        sc -= sc.max(-1, keepdims=True)
        e = np.exp(sc)
        attn = e / e.sum(-1, keepdims=True)
        ctx = np.einsum("hqk,hkd->hqd", attn, V).transpose(1, 0, 2).reshape(S, D)
        out[b] = ctx @ wo + bo
    return out


def kernel(query, key, value, mask, wq, bq, wk, bk, wv, bv, wo, bo):
    global LAST_RESULTS
    query = np.asarray(query, np.float32)
    key = np.asarray(key, np.float32)
    value = np.asarray(value, np.float32)
    mask = np.asarray(mask)
    wq, bq = np.asarray(wq, np.float32), np.asarray(bq, np.float32)
    wk, bk = np.asarray(wk, np.float32), np.asarray(bk, np.float32)
    wv, bv = np.asarray(wv, np.float32), np.asarray(bv, np.float32)
    wo, bo = np.asarray(wo, np.float32), np.asarray(bo, np.float32)

    tril = np.tril(np.ones((S, S), mask.dtype))
    if not all(np.array_equal(mask[b], tril) for b in range(B)):
        return _reference_fallback(query, key, value, mask, wq, bq, wk, bk,
                                   wv, bv, wo, bo)

    with_biases = any(np.any(b_ != 0) for b_ in (bq, bk, bv))

    strip = _mask_strip()
    xT = {}
    for b in range(B):
        xT[("q", b)] = _pretile_x(query[b])
        xT[("k", b)] = _pretile_x(key[b])
        xT[("v", b)] = _pretile_x(value[b])

    in_maps = []
    for core in range(8):
        b, g = core // G, core % G
        cs = slice(g * NG, (g + 1) * NG)
        m = {
            "xqT": xT[("q", b)],
            "xkT": xT[("k", b)],
            "xvT": xT[("v", b)],
            "wq": _pretile_w(wq[:, cs]),
            "wk": _pretile_w(wk[:, cs]),
            "wv": _pretile_w(wv[:, cs]),
            "wo": _pretile_w(wo[cs, :]),
            "mstrip": strip,
            "ident": np.eye(128, dtype=BF16_NP),
        }
        if with_biases:
            m["bq"] = np.ascontiguousarray(bq[cs].reshape(2, 128).T)
            m["bk"] = np.ascontiguousarray(bk[cs].reshape(2, 128).T)
            m["bv"] = np.ascontiguousarray(bv[cs].reshape(2, 128).T)
        in_maps.append(m)

    nc = _get_nc(with_biases)
    res = run_bass_kernel_spmd(nc, in_maps, list(range(8)), trace=TRACE)
    LAST_RESULTS = res

    out = np.empty((B, S, D), np.float32)
    for b in range(B):
        acc = res.results[b * G]["out"].astype(np.float32)
        for g in range(1, G):
            acc = acc + res.results[b * G + g]["out"].astype(np.float32)
        out[b] = acc + bo
    return out


# revision 32
# speedup vs baseline: 1.1701x; 1.0616x over previous
"""Multi-head attention (B=2, S=2048, D=1024, H=16 heads, causal) on 8 TRN2 cores.

Sharding: core i handles batch b=i//4 and head group g=i%4 (4 heads = 256 dims).
Each core computes QKV projections for its head group, causal attention, and a
partial output projection (its 256-dim slice of the contraction). Host sums the
4 partials per batch and adds the output bias.

v3 design (vs v2 at ~168us):
  The HW trace shows PE matmuls never overlap on the array -- cost is simply
  sum(output free size) cycles over all matmuls.  v2's AV matmuls
  (out [65, w] = V|1^T x pp) streamed N=w for each head: 69632 cycles, plus a
  16384-cycle ones-matmul chain to broadcast softmax denominators.  v3 swaps
  the AV operands: per 128-wide q-tile, out[q,65] = pp_tile^T @ [V|ones]
  streams only N=65 per (head, q-tile, k-tile): 35360 cycles, 100% array
  utilization, and the denominator lands in column 64 as a per-PARTITION
  scalar, so normalization is a cheap tensor mul with a [P,1] reciprocal
  (no PE broadcast, no [1,512] single-lane copies).  ctx then comes out
  [q, dims] and a bf16 PE transpose (1 cyc/row, 4096 cycles total) restores
  the [dims, q] layout the output projection needs.
  Other changes:
  - PE warm-up matmuls on a zeroed scratch tile fill the startup DMA window
    so the PE p-state/HAM clock is at full speed when real work arrives
  - startup DMAs use fewer trigger instructions (each costs ~600ns on the
    sync queue) ordered wq, xq, wk, xk, ...
  - last chunk runs norm/transpose/oproj per q-tile as soon as that q-tile's
    AV accumulation stops, shrinking the serial tail
  - two oproj groups of chunk 1 are deferred to chunk 3, whose kt loop is the
    most exp(ACT)-bound, to keep the PE fed there
"""
import sys

import numpy as np

try:
    import concourse.bass as bass  # noqa: F401
except ImportError:
    sys.path.insert(0, "/opt/trn_rl_repo")

import ml_dtypes

import concourse.bass as bass  # noqa: F401
import concourse.mybir as mybir
import concourse.tile as tile
from concourse import bacc
from concourse.bass_utils import run_bass_kernel_spmd

FP32 = mybir.dt.float32
F32R = mybir.dt.float32r
BF16 = mybir.dt.bfloat16
AF = mybir.ActivationFunctionType
BF16_NP = ml_dtypes.bfloat16

B, S, D = 2, 2048, 1024
NH, DK = 16, 64
G = 4              # head groups (cores per batch)
HPG = NH // G      # heads per group = 4
NG = HPG * DK      # dims per group = 256
CH = 512           # q-chunk width
NCH = S // CH      # 4 chunks
NKT = S // 128     # 16 k-tiles
KD = D // 128      # 8 contraction tiles for projections
SCALE = 1.0 / np.sqrt(DK)
VW = 66            # V block stride: 64 dims + ones col + pad (even stride
                   # keeps the DMA-written ones col word-aligned vs the
                   # DVE-written dims -- avoids cross-engine bf16 RMW races)
NWARM = 15         # PE warm-up matmuls bridging the startup DMA window
                   # (first x/w data lands ~11-16us in depending on device
                   # state; a cold PE runs these at ~1GHz, a warm one at 2.4)

TRACE = False          # test harness can set kernel.TRACE = True
LAST_RESULTS = None    # test harness reads kernel.LAST_RESULTS

_NC_CACHE = {}


DEBUG_DUMP = False


def _build_nc(with_biases):
    nc = bacc.Bacc()
    # x inputs and weights arrive pre-tiled in SBUF layout (host rearranges)
    xqT = nc.declare_dram_parameter("xqT", [NCH, 128, KD * CH], BF16, isOutput=False)
    xkT = nc.declare_dram_parameter("xkT", [NCH, 128, KD * CH], BF16, isOutput=False)
    xvT = nc.declare_dram_parameter("xvT", [NCH, 128, KD * CH], BF16, isOutput=False)
    wq = nc.declare_dram_parameter("wq", [128, KD * NG], BF16, isOutput=False)
    wk = nc.declare_dram_parameter("wk", [128, KD * NG], BF16, isOutput=False)
    wv = nc.declare_dram_parameter("wv", [128, KD * NG], BF16, isOutput=False)
    wo = nc.declare_dram_parameter("wo", [128, 2 * D], BF16, isOutput=False)
    mstrip = nc.declare_dram_parameter("mstrip", [128, CH], BF16, isOutput=False)
    ident = nc.declare_dram_parameter("ident", [128, 128], BF16, isOutput=False)
    if with_biases:
        bq = nc.declare_dram_parameter("bq", [128, 2], FP32, isOutput=False)
        bk = nc.declare_dram_parameter("bk", [128, 2], FP32, isOutput=False)
        bv = nc.declare_dram_parameter("bv", [128, 2], FP32, isOutput=False)
    out = nc.declare_dram_parameter("out", [S, D], BF16, isOutput=True)
    if DEBUG_DUMP:
        dbg = {
            nm: nc.declare_dram_parameter(nm, [128, 2 * S], BF16, isOutput=True)
            for nm in ("qd", "kd", "cd")
        }
        dbg["vd"] = nc.declare_dram_parameter("vd", [128, NKT * HPG * VW], BF16,
                                              isOutput=True)

    with tile.TileContext(nc) as tc:
        with (
            tc.tile_pool(name="wpool", bufs=1) as wpool,
            tc.tile_pool(name="cpool", bufs=1) as cpool,
            tc.tile_pool(name="big", bufs=1) as big,
            tc.tile_pool(name="xq", bufs=2) as xqp,
            tc.tile_pool(name="xk", bufs=2) as xkp,
            tc.tile_pool(name="xv", bufs=2) as xvp,
            tc.tile_pool(name="pp", bufs=6) as ppool,
            tc.tile_pool(name="ctp", bufs=2) as ctp,
            tc.tile_pool(name="rbp", bufs=2) as rbp,
            tc.tile_pool(name="ost", bufs=3) as ostp,
            tc.tile_pool(name="ps_x", bufs=2, space="PSUM") as ps_x,
            tc.tile_pool(name="ps_s", bufs=2, space="PSUM") as ps_s,
            tc.tile_pool(name="ps_av", bufs=1, space="PSUM") as ps_av,
        ):
            # ---- resident weights / constants (DMAs emitted lazily below so
            #      Q/K projections can start as early as possible) ----
            wq_sb = wpool.tile([128, KD * NG], BF16, tag="wq")
            wk_sb = wpool.tile([128, KD * NG], BF16, tag="wk")
            wv_sb = wpool.tile([128, KD * NG], BF16, tag="wv")
            wo_sb = wpool.tile([128, 2 * D], BF16, tag="wo")

            def dma_w(w_sb, w_dram, halves=1):
                n = w_sb.shape[1]
                hw = n // halves
                for h in range(halves):
                    nc.sync.dma_start(out=w_sb[:, h * hw:(h + 1) * hw],
                                      in_=w_dram[:, h * hw:(h + 1) * hw])

            mask_sb = cpool.tile([128, CH], BF16, tag="mask")
            ident_sb = cpool.tile([128, 128], BF16, tag="ident")
            wsrc = cpool.tile([128, CH], BF16, tag="wsrc")
            if with_biases:
                bq_sb = cpool.tile([128, 2], FP32, tag="bq")
                bk_sb = cpool.tile([128, 2], FP32, tag="bk")
                bv_sb = cpool.tile([128, 2], FP32, tag="bv")

            # ---- persistent activations ----
            q_sb = [big.tile([128, S], BF16, tag=f"q{m}", name=f"q{m}") for m in range(2)]
            k_sb = [big.tile([128, S], BF16, tag=f"k{m}", name=f"k{m}") for m in range(2)]
            ctx_sb = [big.tile([128, S], BF16, tag=f"ctx{m}", name=f"ctx{m}") for m in range(2)]
            # V: 16 s-tiles x 4 heads x (64 dims + ones col)
            v_sb = big.tile([128, NKT * HPG * VW], BF16, tag="v")

            xt = {}

            def load_x(c, names="qkv", halves=1):
                for nm, pool_, dram_ in (("q", xqp, xqT), ("k", xkp, xkT),
                                         ("v", xvp, xvT)):
                    if nm not in names:
                        continue
                    t_ = pool_.tile([128, KD * CH], BF16, tag="x",
                                    name=f"x{nm}")
                    hw = KD * CH // halves
                    for h in range(halves):
                        nc.sync.dma_start(out=t_[:, h * hw:(h + 1) * hw],
                                          in_=dram_[c, :, h * hw:(h + 1) * hw])
                    xt[(nm, c)] = t_

            def qk_group(c, which):
                # one 128-dim output block of Q^T or K^T for chunk c
                proj, m = divmod(which, 2)
                x_t = xt[("q" if proj == 0 else "k", c)]
                w_sb_ = wq_sb if proj == 0 else wk_sb
                dst = (q_sb if proj == 0 else k_sb)[m]
                pt = ps_x.tile([128, CH], FP32, tag="px", name="pt")
                for kd in range(KD):
                    nc.tensor.matmul(
                        pt[:],
                        lhsT=w_sb_[:, kd * NG + m * 128: kd * NG + m * 128 + 128],
                        rhs=x_t[:, kd * CH: kd * CH + CH],
                        start=(kd == 0), stop=(kd == KD - 1),
                    )
                dstv = dst[:, c * CH:(c + 1) * CH]
                if with_biases:
                    b_sb_ = bq_sb if proj == 0 else bk_sb
                    nc.vector.tensor_scalar_add(dstv, pt[:], b_sb_[:, m:m + 1])
                else:
                    nc.vector.tensor_copy(dstv, pt[:])

            def v_group(c, half):
                # two 128-row s-subtiles of V for chunk c
                x_t = xt[("v", c)]
                pv = ps_x.tile([128, CH], FP32, tag="px", name="pv")
                for ss in (2 * half, 2 * half + 1):
                    col = (ss - 2 * half) * NG
                    for kd in range(KD):
                        nc.tensor.matmul(
                            pv[:, col:col + NG],
                            lhsT=x_t[:, kd * CH + ss * 128:
                                     kd * CH + ss * 128 + 128],
                            rhs=wv_sb[:, kd * NG: kd * NG + NG],
                            start=(kd == 0), stop=(kd == KD - 1),
                        )
                for ss in (2 * half, 2 * half + 1):
                    st = 4 * c + ss
                    col = (ss - 2 * half) * NG
                    dst = v_sb[:, st * HPG * VW: (st + 1) * HPG * VW]
                    dst = dst.rearrange("p (h e) -> p h e", h=HPG)[:, :, 0:64]
                    src = pv[:, col:col + NG].rearrange("p (h e) -> p h e", h=HPG)
                    nc.vector.tensor_copy(dst, src)

            ot_big = {}
            ot_done = {}

            def oproj_group(c, st):
                # partial output projection of one 128-row q block of chunk c;
                # all 4 blocks share one staging tile, DMA'd once per chunk
                r0 = c * CH + st * 128
                if c not in ot_big:
                    ot_big[c] = ostp.tile([128, 4 * D], BF16, tag="ot", name="ot")
                    ot_done[c] = 0
                ot = ot_big[c]
                last = c == NCH - 1
                for mo in range(2):
                    pot = ps_x.tile([128, CH], FP32, tag="px", name="pot")
                    for kk in range(2):
                        nc.tensor.matmul(
                            pot[:],
                            lhsT=ctx_sb[kk][:, r0:r0 + 128],
                            rhs=wo_sb[:, kk * D + mo * CH: kk * D + mo * CH + CH],
                            start=(kk == 0), stop=(kk == 1),
                        )
                    dstc = ot[:, st * D + mo * CH: st * D + mo * CH + CH]
                    if last and mo == 1:
                        # ACT is exp-idle by the last chunk's output stage
                        nc.scalar.copy(dstc, pot[:])
                    else:
                        nc.vector.tensor_copy(dstc, pot[:])
                    if last:
                        # stream each half out as soon as it is staged so the
                        # final DMA (and the exit sequence) starts earlier;
                        # the mo=1 half rides the ACT ring (exp-idle by now)
                        # so both halves fly in parallel
                        eng = nc.scalar if mo == 1 else nc.sync
                        eng.dma_start(
                            out=out[r0:r0 + 128, mo * CH:(mo + 1) * CH],
                            in_=dstc)
                ot_done[c] += 1
                if last:
                    pass
                elif ot_done[c] == 4:
                    nc.sync.dma_start(
                        out=out[c * CH:(c + 1) * CH, :]
                            .rearrange("(i p) m -> p i m", p=128),
                        in_=ot[:].rearrange("p (i m) -> p i m", i=4),
                    )

            def score_phase(c, hp, kt):
                # scores [k=128, q=w] for the head pair, exp on ACT, and the
                # causal mask-mul for diagonal tiles.  Returns the exp tile;
                # its AV fan-out is issued one kt later (software pipeline)
                # so the in-order PE queue never waits on exp.
                j = kt - 4 * c
                mt = hp // 2
                w = CH - 128 * j if j > 0 else CH
                off = CH - w
                sp = ps_s.tile([128, 2 * CH], FP32, tag="sp", name="sp")
                for i in range(2):
                    po = i * 64
                    nc.tensor.matmul(
                        sp[:, i * CH: i * CH + w],
                        lhsT=k_sb[mt][po:po + 64, kt * 128: kt * 128 + 128],
                        rhs=q_sb[mt][po:po + 64, c * CH + off: (c + 1) * CH],
                        start=True, stop=True,
                    )
                pp = ppool.tile([128, 2 * CH], BF16, tag="p", name="pp")
                if w == CH:
                    # contiguous full-width tile: flat AP is cheaper on ACT
                    nc.scalar.activation(pp[:], sp[:], AF.Exp, scale=SCALE)
                else:
                    pview = pp[:].rearrange("p (t x) -> p t x", t=2)[:, :, 0:w]
                    sview = sp[:].rearrange("p (t x) -> p t x", t=2)[:, :, 0:w]
                    nc.scalar.activation(pview, sview, AF.Exp, scale=SCALE)
                return pp

            def mask_mul(pp):
                mview = pp[:].rearrange("p (t x) -> p t x", t=2)[:, :, 0:128]
                nc.vector.tensor_mul(
                    mview, mview,
                    mask_sb[:, None, 0:128].to_broadcast((128, 2, 128)),
                )

            def att_kt(c, hp, kt, pav, started):
                # v7 issue order: scores+exp, then the AV fan for q-tiles
                # strictly past the diagonal (they depend only on exp), then
                # the mask-mul, then the diagonal q-tile's AVs
                j = kt - 4 * c
                pp = score_phase(c, hp, kt)
                if j >= 0:
                    av_phase(c, hp, kt, pav, pp, started, qlo=max(j, 0) + 1)
                    mask_mul(pp)
                    av_phase(c, hp, kt, pav, pp, started, qhi=max(j, 0) + 1)
                else:
                    av_phase(c, hp, kt, pav, pp, started)

            def av_phase(c, hp, kt, pav, pp, started, qlo=0, qhi=4):
                # Swapped AV fan-out: per q-tile ql, out[q,65] = pp^T @ [V|1].
                # The 4 ql blocks are concurrent accumulation groups at
                # disjoint 65-col regions of one bank.  start=True marks the
                # WHOLE bank pending-zero, so only the first kt=0 write per
                # bank sets it; the other kt=0 writes land on pending-zero
                # bytes and overwrite (= implicit start).  skip_group_check:
                # the sim's per-bank one-pending-group bookkeeping can't
                # represent this (stop at kt=4c+ql clears the whole bank's
                # started flag).
                j = kt - 4 * c
                off = 128 * j if j > 0 else 0
                for i in range(2):
                    for ql in range(max(qlo, max(j, 0)), min(qhi, 4)):
                        vcol = (kt * HPG + hp + i) * VW
                        st = kt == 0 and not started[i]
                        started[i] = True
                        nc.tensor.matmul(
                            pav[i][:, ql * 65: ql * 65 + 65],
                            lhsT=pp[:, i * CH + ql * 128 - off:
                                    i * CH + ql * 128 - off + 128],
                            rhs=v_sb[:, vcol:vcol + 65],
                            start=st, stop=(kt == 4 * c + ql),
                            skip_group_check=True,
                        )

            def norm_part1(c, hp, pav, qls):
                # DVE only: reciprocal of the denominator column (a
                # per-partition scalar) and the normalize-mul into a [q, dims]
                # bf16 staging tile.  No PE instructions, so the PE queue
                # flows straight into the next head-pair's scores.
                nq = len(qls)
                q0 = qls[0]
                rb = rbp.tile([128, 2, 4], FP32, tag="rb", name="rb")
                ctxT = ctp.tile([128, CH], BF16, tag="ctxT", name="ctxT")
                for i in range(2):
                    pav_v = pav[i][:].rearrange("p (q e) -> p q e", e=65)
                    nc.vector.reciprocal_approx_fast(
                        out=rb[:, i, q0:q0 + nq, None],
                        in_=pav_v[:, q0:q0 + nq, 64:65])
                for i in range(2):
                    pav_v = pav[i][:].rearrange("p (q e) -> p q e", e=65)
                    dst = ctxT[:].rearrange("p (q i e) -> p q i e", i=2, e=64)
                    nc.vector.tensor_mul(
                        dst[:, q0:q0 + nq, i, :],
                        pav_v[:, q0:q0 + nq, 0:64],
                        rb[:, i, q0:q0 + nq, None].to_broadcast((128, nq, 64)),
                    )
                return ctxT

            def norm_part2(c, hp, ctxT, qls):
                # PE transpose back to [dims, q] + drain into ctx_sb.  The
                # transposes wait on part1's DVE muls, so this is deferred
                # into the NEXT kt loop (the in-order PE queue would
                # otherwise stall the next head-pair's scores behind them).
                mt = hp // 2
                nq = len(qls)
                q0 = qls[0]
                pt = ps_x.tile([128, 2 * CH], BF16, tag="px", name="ptr")
                for ql in qls:
                    nc.tensor.transpose(pt[:, ql * 128:(ql + 1) * 128],
                                        ctxT[:, ql * 128:(ql + 1) * 128],
                                        ident_sb[:])
                dst = ctx_sb[mt][:, c * CH + q0 * 128: c * CH + (q0 + nq) * 128]
                src = pt[:, q0 * 128:(q0 + nq) * 128]
                if with_biases:
                    nc.vector.tensor_scalar_add(dst, src, bv_sb[:, mt:mt + 1])
                else:
                    nc.vector.tensor_copy(dst, src)

            # ---- schedule ----
            # Warm-up: matmuls on a zeroed scratch tile ramp the PE clock
            # while the startup DMAs stream in.
            vview = v_sb[:].rearrange("p (t e) -> p t e", e=VW)[:, :, 64:65]
            nc.gpsimd.memset(vview, 1.0)
            if DEBUG_DUMP:
                # the VW pad col is never written; init it so the sim's
                # uninitialized-read check passes on the full-v_sb dump
                nc.gpsimd.memset(
                    v_sb[:].rearrange("p (t e) -> p t e", e=VW)[:, :, 65:66], 0.0)
            nc.gpsimd.memset(wsrc[:], 0.0)
            for _ in range(NWARM):
                pw = ps_x.tile([128, CH], FP32, tag="px", name="pw")
                nc.tensor.matmul(pw[:], lhsT=wsrc[:, 0:128], rhs=wsrc[:],
                                 start=True, stop=True)
            # Startup: two HWDGE rings (SP + ACT) stream the startup set in
            # parallel; the ACT ring is idle until the first exp (~16us).
            # Ring order keeps wq + both xq halves landing first so the
            # Q-projections start right as the warm-up runs out.
            # All bulk transfers ride the sync ring (the ACT ring measured
            # ~3x slower); only the tiny mask/ident constants go on the ACT
            # ring so they don't displace the critical startup stream.
            # A 4-byte dummy absorbs any one-time ring-init latency first.
            nc.sync.dma_start(out=ident_sb[0:1, 0:2], in_=ident[0:1, 0:2])
            dma_w(wq_sb, wq, halves=2)
            load_x(0, "q", halves=4)
            nc.scalar.dma_start(out=mask_sb[:], in_=mstrip[:])
            nc.scalar.dma_start(out=ident_sb[:], in_=ident[:])
            if with_biases:
                nc.sync.dma_start(out=bq_sb[:], in_=bq[:])
                nc.sync.dma_start(out=bk_sb[:], in_=bk[:])
                nc.sync.dma_start(out=bv_sb[:], in_=bv[:])
            nc.scalar.dma_start(out=wv_sb[:], in_=wv[:])
            qk_group(0, 0)
            qk_group(0, 1)
            dma_w(wk_sb, wk)
            load_x(0, "k", halves=2)
            qk_group(0, 2)
            qk_group(0, 3)
            load_x(0, "v", halves=2)
            for half in range(2):
                v_group(0, half)
            dma_w(wo_sb, wo)

            # Chunk loop. Work that later chunks depend on is deferred as far
            # as its dependencies allow, so the bigger (attention-heavy, exp-
            # bound) chunks get more PE filler between their matmuls:
            #   - Q(c+1) is the only projection needed before chunk c+1 starts
            #   - K(c)/V(c) are only needed by chunk c's last 4 kt tiles
            #     (the diagonal), so they are emitted inside chunk c itself
            #   - oproj(c-1) fills chunk c; two oproj(1) groups are carried
            #     into chunk 3 (the most exp-bound kt loop)
            #   - each norm's part2 (PE transposes) is deferred to iteration 1
            #     of the NEXT kt loop so it never stalls the in-order PE queue
            carry = []
            part2_q = []
            for c in range(NCH):
                if c < NCH - 1:
                    load_x(c + 1)
                niters = 2 * (4 * c + 4)
                early = []
                if c >= 1:
                    if c >= 2:
                        early += [lambda w=w_: qk_group(c, w) for w_ in (2, 3)]
                    early += [lambda h=h_: v_group(c, h) for h_ in range(2)]
                late = []
                if c < NCH - 1:
                    late += [lambda w=w_: qk_group(c + 1, w) for w_ in (0, 1)]
                if c >= 1:
                    ops = [lambda s=s_, cc=c - 1: oproj_group(cc, s)
                           for s_ in range(4)]
                    if c == NCH - 1:
                        late += ops + carry
                    elif c == NCH - 2:
                        # defer two oproj groups an extra chunk: the last
                        # chunk has the deepest exp deficit
                        late += ops[0:2]
                        carry = ops[2:]
                    else:
                        late += ops
                else:
                    late += [lambda w=w_: qk_group(1, w) for w_ in (2, 3)]
                sched = [[] for _ in range(niters)]
                # early groups: finish before iteration 4c of the hp=0 loop
                span_e = max(1, 4 * c - 1)
                for gi, g in enumerate(early):
                    pos = (gi * span_e) // (len(early) - 1) if len(early) > 1 else 0
                    sched[min(span_e, pos)].append(g)
                # pin the first two late groups at the hp0->hp2 and
                # hp2->next-chunk boundary iterations: the next loop's first
                # scores wait there for the old hp's last exps to free their
                # psum tiles, so the PE needs filler
                # (for the last chunk keep the final iterations clear -- the
                # per-q-tile tail chain owns them); when groups are plentiful
                # put a second one at the hp0->hp2 boundary (the stall there
                # is ~2 group-lengths deep)
                pins = [4 * c + 3, niters - 1 if c < NCH - 1 else niters - 6]
                if len(late) >= 5:
                    pins.append(4 * c + 3)
                for gi, g in enumerate(late):
                    if gi < len(pins) and c >= 1:
                        pos = pins[gi]
                    else:
                        pos = min(niters - 1,
                                  (gi + 1) * niters // (len(late) + 1))
                    sched[pos].append(g)
                it = 0
                last_c = c == NCH - 1
                for hp in (0, 2):
                    pav = [ps_av.tile([128, 260], FP32, tag=f"av{i}",
                                      name=f"pav{i}")
                           for i in range(2)]
                    started = [False, False]
                    pend = None
                    nkt = 4 * c + 4
                    for kt in range(nkt):
                        att_kt(c, hp, kt, pav, started)
                        if kt == 1:
                            for g in part2_q:
                                g()
                            part2_q = []
                        if pend is not None:
                            # tail pipeline: transpose + oproj of the previous
                            # q-tile, one iteration after its norm part1
                            ctxT_, ql_ = pend
                            norm_part2(c, hp, ctxT_, [ql_])
                            oproj_group(c, ql_)
                            pend = None
                        for g in sched[it]:
                            g()
                        it += 1
                        if last_c and hp == 2 and kt >= 4 * c:
                            # per-q-tile tail: q-tile ql's AV accumulation
                            # stopped at kt == 4c+ql; normalize it now
                            ql = kt - 4 * c
                            pend = (norm_part1(c, hp, pav, [ql]), ql)
                    if last_c and hp == 2:
                        if pend is not None:
                            ctxT_, ql_ = pend
                            norm_part2(c, hp, ctxT_, [ql_])
                            oproj_group(c, ql_)
                    else:
                        ctxT = norm_part1(c, hp, pav, [0, 1, 2, 3])
                        if with_biases:
                            part2_q.append(
                                lambda cc=c, hh=hp, ct=ctxT: norm_part2(
                                    cc, hh, ct, [0, 1, 2, 3]))
                        else:
                            # XBAR-transposing DMA does [q,dims]->[dims,q]
                            # per 128-slab entirely off the PE/DVE; the
                            # consumer (oproj) runs a chunk later, hiding
                            # the DMA latency
                            mt = hp // 2
                            nc.sync.dma_start_transpose(
                                out=ctx_sb[mt][:, c * CH:(c + 1) * CH]
                                    .rearrange("p (i j) -> p i j", j=128),
                                in_=ctxT[:])

            if DEBUG_DUMP:
                for nm, pair in (("qd", q_sb), ("kd", k_sb), ("cd", ctx_sb)):
                    for m in range(2):
                        nc.sync.dma_start(out=dbg[nm][:, m * S:(m + 1) * S],
                                          in_=pair[m][:])
                nc.sync.dma_start(out=dbg["vd"][:], in_=v_sb[:])

    nc.compile()
    return nc


def _get_nc(with_biases):
    if with_biases not in _NC_CACHE:
        _NC_CACHE[with_biases] = _build_nc(with_biases)
    return _NC_CACHE[with_biases]


def _pretile_x(x):
    # [S, D] activation -> [NCH, 128, KD*CH] bf16 in SBUF layout:
    # out[c, p, kd*CH + s] = x[c*CH + s, kd*128 + p]
    xt = np.ascontiguousarray(x.T)                       # [D, S]
    t = xt.reshape(KD, 128, NCH, CH).transpose(2, 1, 0, 3)
    return np.ascontiguousarray(t.reshape(NCH, 128, KD * CH)).astype(BF16_NP)


def _pretile_w(w):
    # [D, n] weight -> [128, KD*n] bf16: out[p, kd*n + j] = w[kd*128 + p, j]
    n = w.shape[1]
    t = w.reshape(KD if w.shape[0] == D else 2, 128, n).transpose(1, 0, 2)
    return np.ascontiguousarray(t.reshape(128, -1)).astype(BF16_NP)


def _mask_strip():
    # strip[p, y] = 1.0 iff y >= p; with the causal sub-range offset applied
    # to the q-columns, every diagonal k-tile masks with strip[:, 0:w]
    y = np.arange(CH)[None, :]
    p = np.arange(128)[:, None]
    return (y >= p).astype(BF16_NP)


def _reference_fallback(query, key, value, mask, wq, bq, wk, bk, wv, bv, wo, bo):
    out = np.empty((B, S, D), np.float32)
    for b in range(B):
        Q = (query[b] @ wq + bq).reshape(S, NH, DK).transpose(1, 0, 2)
        K = (key[b] @ wk + bk).reshape(S, NH, DK).transpose(1, 0, 2)
        V = (value[b] @ wv + bv).reshape(S, NH, DK).transpose(1, 0, 2)
        sc = np.einsum("hqd,hkd->hqk", Q, K).astype(np.float32) / np.sqrt(DK)
        sc = np.where(mask[b][None] == 0, -1.0e9, sc)
        sc -= sc.max(-1, keepdims=True)
        e = np.exp(sc)
        attn = e / e.sum(-1, keepdims=True)
        ctx = np.einsum("hqk,hkd->hqd", attn, V).transpose(1, 0, 2).reshape(S, D)
        out[b] = ctx @ wo + bo
    return out


def kernel(query, key, value, mask, wq, bq, wk, bk, wv, bv, wo, bo):
    global LAST_RESULTS
    query = np.asarray(query, np.float32)
    key = np.asarray(key, np.float32)
    value = np.asarray(value, np.float32)
    mask = np.asarray(mask)
    wq, bq = np.asarray(wq, np.float32), np.asarray(bq, np.float32)
    wk, bk = np.asarray(wk, np.float32), np.asarray(bk, np.float32)
    wv, bv = np.asarray(wv, np.float32), np.asarray(bv, np.float32)
    wo, bo = np.asarray(wo, np.float32), np.asarray(bo, np.float32)

    tril = np.tril(np.ones((S, S), mask.dtype))
    if not all(np.array_equal(mask[b], tril) for b in range(B)):
        return _reference_fallback(query, key, value, mask, wq, bq, wk, bk,
                                   wv, bv, wo, bo)

    with_biases = any(np.any(b_ != 0) for b_ in (bq, bk, bv))

    strip = _mask_strip()
    xT = {}
    for b in range(B):
        xT[("q", b)] = _pretile_x(query[b])
        xT[("k", b)] = _pretile_x(key[b])
        xT[("v", b)] = _pretile_x(value[b])

    in_maps = []
    for core in range(8):
        b, g = core // G, core % G
        cs = slice(g * NG, (g + 1) * NG)
        m = {
            "xqT": xT[("q", b)],
            "xkT": xT[("k", b)],
            "xvT": xT[("v", b)],
            "wq": _pretile_w(wq[:, cs]),
            "wk": _pretile_w(wk[:, cs]),
            "wv": _pretile_w(wv[:, cs]),
            "wo": _pretile_w(wo[cs, :]),
            "mstrip": strip,
            "ident": np.eye(128, dtype=BF16_NP),
        }
        if with_biases:
            m["bq"] = np.ascontiguousarray(bq[cs].reshape(2, 128).T)
            m["bk"] = np.ascontiguousarray(bk[cs].reshape(2, 128).T)
            m["bv"] = np.ascontiguousarray(bv[cs].reshape(2, 128).T)
        in_maps.append(m)

    nc = _get_nc(with_biases)
    res = run_bass_kernel_spmd(nc, in_maps, list(range(8)), trace=TRACE)
    LAST_RESULTS = res

    out = np.empty((B, S, D), np.float32)
    for b in range(B):
        acc = res.results[b * G]["out"].astype(np.float32)
        for g in range(1, G):
            acc = acc + res.results[b * G + g]["out"].astype(np.float32)
        out[b] = acc + bo
    return out
